# revision 1
# baseline (speedup 1.0000x reference)
"""Multi-head attention (B=2, S=2048, D=1024, H=16) on 8 TRN2 NeuronCores, v2.

Sharding: data-parallel over batch (2) x tensor-parallel over head groups
(4 groups of 4 heads).  Core c = (b = c // 4, g = c % 4).

v2 design (act-engine-centric):
  - Act engine does ONLY exp (128 x [128,1024] chunks, ~140us = bottleneck).
  - scores computed [k, q] in fp32r (qT/kT fp32), exp -> pt bf16.
  - PV in [q, dh] orientation: lhsT = pt chunk [k,128q], rhs = v_ext [k,65]
    (64 v cols + ones col -> denominators land in pv col 64). Halves PV
    tensor rows vs the [dh, q] orientation.
  - normalize = DVE reciprocal [128,8] + per-partition tensor_scalar_mul.
  - PE transpose (identity) puts normalized attn back into aT [d, q] layout;
    v-bias is folded into the transpose-copy (per-partition bias there).
  - projections / scores share one PSUM pool so phases overlap; out-proj of
    q-group i is software-pipelined into attention of group i+1.
"""

import os
import sys
import types
from contextlib import ExitStack

import numpy as np

D = 1024
S = 2048
C = 256          # head cols per core (4 heads x 64)
DH = 64
NH = 4           # heads per core
QG = 1024        # q-group width
NQG = S // QG    # 2
NST = S // 128   # 16 seq tiles
NSB = QG // 128  # 8 q-subtiles per group

_CACHE = {}


def _install_ntff_shim():
    try:
        import antenv.axon_hooks  # noqa: F401
        return
    except ImportError:
        pass
    try:
        from trn_agent_boot.trn_boot import _ntff_profile_via_ctypes
        hook = _ntff_profile_via_ctypes('/opt/axon/libaxon_pjrt.so')
    except Exception:
        hook = None
    mod = types.ModuleType('antenv.axon_hooks')
    mod.get_axon_ntff_profile_hook = lambda: hook
    mod.set_axon_ntff_profile_hook = lambda h: None
    sys.modules['antenv.axon_hooks'] = mod


def build_nc():
    import concourse.bacc as bacc
    import concourse.mybir as mybir
    import concourse.tile as tile
    from concourse.bass import ts, ds

    F32 = mybir.dt.float32
    F32R = mybir.dt.float32r
    BF16 = mybir.dt.bfloat16
    ACT = mybir.ActivationFunctionType

    nc = bacc.Bacc("TRN2", target_bir_lowering=False, debug=False)
    xT = nc.dram_tensor("xT", [D, S], BF16, kind="ExternalInput")
    wq = nc.dram_tensor("wq", [D, C], BF16, kind="ExternalInput")
    wk = nc.dram_tensor("wk", [D, C], BF16, kind="ExternalInput")
    wv = nc.dram_tensor("wv", [D, C], BF16, kind="ExternalInput")
    wo = nc.dram_tensor("wo", [C, D], F32R, kind="ExternalInput")
    bqk = nc.dram_tensor("bqk", [128, 4], F32, kind="ExternalInput")
    bvb = nc.dram_tensor("bvb", [128, 2], F32, kind="ExternalInput")
    ident = nc.dram_tensor("ident", [128, 128], F32R, kind="ExternalInput")
    yT = nc.dram_tensor("yT", [D, S], F32, kind="ExternalOutput")

    with tile.TileContext(nc) as tc, ExitStack() as ctx:
        consts = ctx.enter_context(tc.tile_pool(name="consts", bufs=1))
        sbw = ctx.enter_context(tc.tile_pool(name="weights", bufs=1))
        sbx = ctx.enter_context(tc.tile_pool(name="xsb", bufs=1))
        sbqkv = ctx.enter_context(tc.tile_pool(name="qkv", bufs=1))
        sbpt = ctx.enter_context(tc.tile_pool(name="ptp", bufs=3))
        sbat = ctx.enter_context(tc.tile_pool(name="atn", bufs=2))
        sbnrm = ctx.enter_context(tc.tile_pool(name="nrm", bufs=2))
        sby = ctx.enter_context(tc.tile_pool(name="ysb", bufs=4))
        # PSUM: sc 2x[128,1024] = 4 banks, pv 1x[128,520] = 2 banks,
        #       tr 1x[128,128] = 1 bank, yp 1x[128,512] = 1 bank -> 8 total
        scp = ctx.enter_context(tc.tile_pool(name="psc", bufs=2, space="PSUM"))
        pvp = ctx.enter_context(tc.tile_pool(name="ppv", bufs=1, space="PSUM"))
        trp = ctx.enter_context(tc.tile_pool(name="ptr", bufs=1, space="PSUM"))
        ypp = ctx.enter_context(tc.tile_pool(name="pyp", bufs=1, space="PSUM"))

        # ---- constants ----
        bqk_sb = consts.tile([128, 4], F32, tag="bqk", name="bqk_sb")
        nc.sync.dma_start(bqk_sb[:], bqk[:, :])
        bvb_sb = consts.tile([128, 2], F32, tag="bvb", name="bvb_sb")
        nc.sync.dma_start(bvb_sb[:], bvb[:, :])
        id_sb = consts.tile([128, 128], F32R, tag="ident", name="id_sb")
        nc.sync.dma_start(id_sb[:], ident[:, :])

        # ---- input DMAs (ordered: wk, xT blocks, wv, wq, wo) ----
        wk_sb = [sbw.tile([128, C], BF16, tag=f"wk{i}", name=f"wk{i}")
                 for i in range(8)]
        wv_sb = [sbw.tile([128, C], BF16, tag=f"wv{i}", name=f"wv{i}")
                 for i in range(8)]
        wq_sb = [sbw.tile([128, C], BF16, tag=f"wq{i}", name=f"wq{i}")
                 for i in range(8)]
        xt_sb = [sbx.tile([128, S], BF16, tag=f"xt{i}", name=f"xt{i}")
                 for i in range(8)]
        for i in range(8):
            nc.sync.dma_start(wk_sb[i][:], wk[ts(i, 128), :])
        for nb in range(4):
            for i in range(8):
                nc.sync.dma_start(xt_sb[i][:, ts(nb, 512)],
                                  xT[ts(i, 128), ts(nb, 512)])
        for i in range(8):
            nc.sync.dma_start(wv_sb[i][:], wv[ts(i, 128), :])
        for i in range(8):
            nc.sync.dma_start(wq_sb[i][:], wq[ts(i, 128), :])
        wo_sb = []
        for i in range(2):
            t = sbw.tile([128, D], F32R, tag=f"wo{i}", name=f"wo{i}")
            nc.sync.dma_start(t[:], wo[ts(i, 128), :])
            wo_sb.append(t)

        # ---- persistent activations ----
        qT_sb = [sbqkv.tile([128, S], F32R, tag=f"qT{i}", name=f"qT{i}")
                 for i in range(2)]
        kT_sb = [sbqkv.tile([128, S], F32R, tag=f"kT{i}", name=f"kT{i}")
                 for i in range(2)]
        v_sb = [sbqkv.tile([128, NH * 65], BF16, tag=f"v{i}", name=f"v{i}")
                for i in range(NST)]
        aT_sb = [sbqkv.tile([128, S], F32R, tag=f"aT{i}", name=f"aT{i}")
                 for i in range(2)]

        # ones columns of v_ext (col 65h+64 = 1.0)
        for st in range(NST):
            v3 = v_sb[st][:].rearrange("p (h e) -> p h e", e=65)
            nc.gpsimd.memset(v3[:, :, 64:65], 1.0)

        # ---- emission helpers ----
        def proj_qk(which, mt, half, pool=None):
            """one [128,1024] stripe of q/k projection -> qT/kT (2 psum tiles)."""
            w_sb, dsts, bcol = {
                "q": (wq_sb, qT_sb, 0), "k": (wk_sb, kT_sb, 2)}[which]
            for h2 in range(2):
                nb = half * 2 + h2
                if pool is None:
                    pj = ypp.tile([128, 512], F32, tag="yp", name="pj")
                else:
                    pj = pool.tile([128, 512], F32, tag="sc", name="pj")
                for kt in range(8):
                    nc.tensor.matmul(
                        pj[:],
                        lhsT=w_sb[kt][:, ts(mt, 128)],
                        rhs=xt_sb[kt][:, ts(nb, 512)],
                        start=(kt == 0), stop=(kt == 7),
                    )
                nc.vector.tensor_scalar_add(
                    dsts[mt][:, ts(nb, 512)], pj[:],
                    bqk_sb[:, bcol + mt:bcol + mt + 1])

        def proj_v(st):
            """one seq-tile of v projection -> v_sb[st] (no bias)."""
            vp = ypp.tile([128, C], F32, tag="yp", name="vp_pj")
            for kt in range(8):
                nc.tensor.matmul(
                    vp[:],
                    lhsT=xt_sb[kt][:, ts(st, 128)],
                    rhs=wv_sb[kt][:],
                    start=(kt == 0), stop=(kt == 7),
                )
            v3 = v_sb[st][:].rearrange("p (h e) -> p h e", e=65)
            nc.vector.tensor_copy(
                v3[:, :, 0:64],
                vp[:].rearrange("p (h e) -> p h e", e=64))

        def transpose_at(pair, qg, qt, at_tile):
            """attn [128q,128d] -> aT[pair][:, qg*QG+qt*128 ...] (+v bias)."""
            tr = trp.tile([128, 128], F32R, tag="tr", name="tr")
            nc.tensor.transpose(tr[:], at_tile[:], id_sb[:])
            nc.vector.tensor_scalar_add(
                aT_sb[pair][:, ds(qg * QG + qt * 128, 128)], tr[:],
                bvb_sb[:, pair:pair + 1])

        def out_proj_mt(nb, mt, tail=False):
            pool = scp if tail else ypp
            yp = pool.tile([128, 512], F32, tag="sc" if tail else "yp",
                           name="yp")
            for p in range(2):
                nc.tensor.matmul(
                    yp[:],
                    lhsT=wo_sb[p][:, ts(mt, 128)],
                    rhs=aT_sb[p][:, ts(nb, 512)],
                    start=(p == 0), stop=(p == 1),
                )
            yt = sby.tile([128, 512], F32, tag="yt", name="yt")
            nc.vector.tensor_copy(yt[:], yp[:])
            nc.sync.dma_start(yT[ts(mt, 128), ts(nb, 512)], yt[:])

        # ---- interleave schedule ----
        # slots[(qg, h)][kt] -> list of thunks emitted after the scores mms
        # of iteration kt (PE program order), before exp/pv.
        slots = {(qg, h): {} for qg in range(NQG) for h in range(NH)}

        def add_slot(qg, h, kt, fn):
            slots[(qg, h)].setdefault(kt, []).append(fn)

        # v-proj st4..15 into qg0/h0 kt0..11
        for j, st in enumerate(range(4, NST)):
            add_slot(0, 0, j, (lambda st=st: proj_v(st)))
        # k-proj mt1 halves into qg0/h0 kt12..15 (needed by h2)
        add_slot(0, 0, 12, lambda: proj_qk("k", 1, 0))
        add_slot(0, 0, 14, lambda: proj_qk("k", 1, 1))
        # q-proj half1 (qg1 cols) into qg0/h1 (needed by qg1)
        add_slot(0, 1, 0, lambda: proj_qk("q", 0, 1))
        add_slot(0, 1, 4, lambda: proj_qk("q", 1, 1))

        # per-(qg,pair) attn tiles, filled by norm, consumed by transpose
        attn_tiles = {}

        def norm_pair_writes(qg, h, pva, pvb):
            pair = h // 2
            if (qg, pair) not in attn_tiles:
                attn_tiles[(qg, pair)] = [
                    sbat.tile([128, 128], F32R, tag=f"at{qt}", name=f"at{qt}")
                    for qt in range(NSB)]
            tiles = attn_tiles[(qg, pair)]
            col = 64 * (h % 2)
            # single copy per bank releases the PSUM WAR fast; recip/muls
            # then work off SBUF without blocking the next head's pv mms
            pvc = sbnrm.tile([128, 2, 4, 65], F32, tag="pvc", name="pvc")
            nc.vector.tensor_copy(
                pvc[:, 0], pva[:].rearrange("p (s e) -> p s e", e=65))
            nc.vector.tensor_copy(
                pvc[:, 1], pvb[:].rearrange("p (s e) -> p s e", e=65))
            recip = sbnrm.tile([128, 8], F32, tag="rc", name="rc")
            pc3 = pvc[:].rearrange("p a s e -> p (a s) e")
            nc.vector.reciprocal(recip[:], pc3[:, :, 64])
            for qs in range(NSB):
                nc.vector.tensor_scalar_mul(
                    tiles[qs][:, col:col + 64],
                    pc3[:, qs, 0:64],
                    recip[:, qs:qs + 1])

        def attn_head(qg, h):
            """software-pipelined: iteration j emits scores(j), pv(j-2),
            exp(j-1) so the act engine runs back-to-back."""
            pair, poff = h // 2, 64 * (h % 2)
            qt_, kt_ = qT_sb[pair], kT_sb[pair]
            # one accumulation group per PSUM bank: 4 q-subtiles per tile
            pva = pvp.tile([128, 4 * 65], F32, tag="pva", name="pva")
            pvb = pvp.tile([128, 4 * 65], F32, tag="pvb", name="pvb")
            isl = slots[(qg, h)]
            pts, scs = {}, {}
            for j in range(NST + 2):
                # exp FIRST in program order: the act engine's coalesced
                # PE-semaphore threshold then only covers work finished a
                # full period ago, so exps run back-to-back.
                kt1 = j - 1
                if 0 <= kt1 < NST:
                    pt = sbpt.tile([128, QG], BF16, tag="pt", name="pt")
                    pts[kt1] = pt
                    nc.scalar.activation(pt[:], scs.pop(kt1)[:], ACT.Exp)
                if j < NST:
                    sc = scp.tile([128, QG], F32, tag="sc", name="sc_at")
                    scs[j] = sc
                    for qb in range(2):
                        nc.tensor.matmul(
                            sc[:, ts(qb, 512)],
                            lhsT=kt_[poff:poff + 64, ts(j, 128)],
                            rhs=qt_[poff:poff + 64,
                                    ds(qg * QG + qb * 512, 512)],
                            start=True, stop=True,
                        )
                    for fn in isl.get(j, ()):
                        fn()
                kt2 = j - 2
                if kt2 >= 0:
                    ptt = pts.pop(kt2)
                    for qs in range(NSB):
                        pvt = pva if qs < 4 else pvb
                        nc.tensor.matmul(
                            pvt[:, ds((qs % 4) * 65, 65)],
                            lhsT=ptt[:, ts(qs, 128)],
                            rhs=v_sb[kt2][:, ds(65 * h, 65)],
                            start=(kt2 == 0 and qs % 4 == 0),
                            stop=(kt2 == NST - 1 and qs % 4 == 3),
                        )
            norm_pair_writes(qg, h, pva, pvb)

        # ---- phase 1: initial projections (k via the idle sc pool so the
        #      lead-in isn't serialized on the single yp buffer) ----
        proj_qk("k", 0, 0, pool=scp)
        proj_qk("k", 0, 1, pool=scp)
        for st in range(4):
            proj_v(st)
        proj_qk("q", 0, 0)
        proj_qk("q", 1, 0)

        # ---- phase 2: attention + pipelined transposes / out-proj ----
        # transposes of (qg, pair) go into slots of the following head pair;
        # out-proj of qg into slots of qg+1; tail handled explicitly.
        for qg in range(NQG):
            # transposes of previous group's pair1
            if qg > 0:
                for qt in range(NSB):
                    add_slot(qg, 0, qt, (lambda qt=qt, qg=qg:
                             transpose_at(1, qg - 1, qt,
                                          attn_tiles[(qg - 1, 1)][qt])))
                # out-proj of previous group spread over h1/h2 slots
                for j, (nb, mt) in enumerate(
                        [(2 * (qg - 1) + b, m) for b in range(2)
                         for m in range(8)]):
                    add_slot(qg, 1 + j // 16, j % 16,
                             (lambda nb=nb, mt=mt: out_proj_mt(nb, mt)))
            for h in range(NH):
                # transposes of pair0 go into h2 slots
                if h == 2:
                    for qt in range(NSB):
                        add_slot(qg, 2, qt, (lambda qt=qt, qg=qg:
                                 transpose_at(0, qg, qt,
                                              attn_tiles[(qg, 0)][qt])))
                attn_head(qg, h)

        # ---- tail: transposes of (qg1, pair1) + out-proj of qg1 ----
        qg = NQG - 1
        for half in range(2):
            for qt in range(half * 4, half * 4 + 4):
                transpose_at(1, qg, qt, attn_tiles[(qg, 1)][qt])
            for mt in range(8):
                out_proj_mt(2 * qg + half, mt, tail=True)

    nc.compile()
    return nc


def make_in_maps(x, Wq, bq, Wk, bk, Wv, bv, Wo):
    """Shard full inputs into 8 per-core input maps."""
    import ml_dtypes
    BF = ml_dtypes.bfloat16
    scale = np.float32(1.0 / np.sqrt(DH))
    xT = [np.ascontiguousarray(x[b].T).astype(BF) for b in range(2)]
    ident = np.eye(128, dtype=np.float32)
    in_maps = []
    for c in range(8):
        b, g = c // 4, c % 4
        sl = slice(C * g, C * (g + 1))
        bq_g = (bq[sl] * scale).reshape(2, 128).T
        bk_g = bk[sl].reshape(2, 128).T
        in_maps.append({
            "xT": xT[b],
            "wq": (np.ascontiguousarray(Wq[:, sl]) * scale).astype(BF),
            "wk": np.ascontiguousarray(Wk[:, sl]).astype(BF),
            "wv": np.ascontiguousarray(Wv[:, sl]).astype(BF),
            "wo": np.ascontiguousarray(Wo[sl, :]).astype(np.float32),
            "bqk": np.ascontiguousarray(
                np.concatenate([bq_g, bk_g], axis=1)).astype(np.float32),
            "bvb": np.ascontiguousarray(
                bv[sl].reshape(2, 128).T).astype(np.float32),
            "ident": ident,
        })
    return in_maps


def kernel(x, Wq, bq, Wk, bk, Wv, bv, Wo, bo):
    if os.environ.get("JAX_PLATFORMS") and \
            "axon" not in os.environ["JAX_PLATFORMS"]:
        os.environ.pop("JAX_PLATFORMS")
    trace = bool(os.environ.get("KERNEL_TRACE"))
    if trace:
        _install_ntff_shim()
    from concourse import bass_utils

    x = np.asarray(x, dtype=np.float32)
    in_maps = make_in_maps(
        x, np.asarray(Wq), np.asarray(bq), np.asarray(Wk), np.asarray(bk),
        np.asarray(Wv), np.asarray(bv), np.asarray(Wo))

    if "nc" not in _CACHE:
        _CACHE["nc"] = build_nc()
    res = bass_utils.run_bass_kernel_spmd(
        _CACHE["nc"], in_maps, core_ids=list(range(8)), trace=trace)
    _CACHE["exec_time_ns"] = res.exec_time_ns

    bo = np.asarray(bo, dtype=np.float32)
    out = np.empty((2, S, D), dtype=np.float32)
    for b in range(2):
        acc = res.results[4 * b]["yT"].copy()
        for g in range(1, 4):
            acc += res.results[4 * b + g]["yT"]
        out[b] = acc.T + bo
    return out



# revision 2
# speedup vs baseline: 1.0622x; 1.0622x over previous
"""Multi-head attention (B=2, S=2048, D=1024, H=16) on 8 TRN2 NeuronCores, v2.

Sharding: data-parallel over batch (2) x tensor-parallel over head groups
(4 groups of 4 heads).  Core c = (b = c // 4, g = c % 4).

v2 design (act-engine-centric):
  - Act engine does ONLY exp (128 x [128,1024] chunks, ~140us = bottleneck).
  - scores computed [k, q] in fp32r (qT/kT fp32), exp -> pt bf16.
  - PV in [q, dh] orientation: lhsT = pt chunk [k,128q], rhs = v_ext [k,65]
    (64 v cols + ones col -> denominators land in pv col 64). Halves PV
    tensor rows vs the [dh, q] orientation.
  - normalize = DVE reciprocal [128,8] + per-partition tensor_scalar_mul.
  - PE transpose (identity) puts normalized attn back into aT [d, q] layout;
    v-bias is folded into the transpose-copy (per-partition bias there).
  - projections / scores share one PSUM pool so phases overlap; out-proj of
    q-group i is software-pipelined into attention of group i+1.
"""

import os
import sys
import types
from contextlib import ExitStack

import numpy as np

D = 1024
S = 2048
C = 256          # head cols per core (4 heads x 64)
DH = 64
NH = 4           # heads per core
QG = 1024        # q-group width
NQG = S // QG    # 2
NST = S // 128   # 16 seq tiles
NSB = QG // 128  # 8 q-subtiles per group

_CACHE = {}


def _install_ntff_shim():
    try:
        import antenv.axon_hooks  # noqa: F401
        return
    except ImportError:
        pass
    try:
        from trn_agent_boot.trn_boot import _ntff_profile_via_ctypes
        hook = _ntff_profile_via_ctypes('/opt/axon/libaxon_pjrt.so')
    except Exception:
        hook = None
    mod = types.ModuleType('antenv.axon_hooks')
    mod.get_axon_ntff_profile_hook = lambda: hook
    mod.set_axon_ntff_profile_hook = lambda h: None
    sys.modules['antenv.axon_hooks'] = mod


def build_nc():
    import concourse.bacc as bacc
    import concourse.mybir as mybir
    import concourse.tile as tile
    from concourse.bass import ts, ds

    F32 = mybir.dt.float32
    F32R = mybir.dt.float32r
    BF16 = mybir.dt.bfloat16
    ACT = mybir.ActivationFunctionType

    nc = bacc.Bacc("TRN2", target_bir_lowering=False, debug=False)
    xT = nc.dram_tensor("xT", [D, S], BF16, kind="ExternalInput")
    wq = nc.dram_tensor("wq", [D, C], BF16, kind="ExternalInput")
    wk = nc.dram_tensor("wk", [D, C], BF16, kind="ExternalInput")
    wv = nc.dram_tensor("wv", [D, C], BF16, kind="ExternalInput")
    wo = nc.dram_tensor("wo", [C, D], F32R, kind="ExternalInput")
    bqk = nc.dram_tensor("bqk", [128, 4], F32, kind="ExternalInput")
    bvb = nc.dram_tensor("bvb", [128, 2], F32, kind="ExternalInput")
    ident = nc.dram_tensor("ident", [128, 128], F32R, kind="ExternalInput")
    yT = nc.dram_tensor("yT", [D, S], F32, kind="ExternalOutput")

    with tile.TileContext(nc) as tc, ExitStack() as ctx:
        consts = ctx.enter_context(tc.tile_pool(name="consts", bufs=1))
        sbw = ctx.enter_context(tc.tile_pool(name="weights", bufs=1))
        sbx = ctx.enter_context(tc.tile_pool(name="xsb", bufs=1))
        sbqkv = ctx.enter_context(tc.tile_pool(name="qkv", bufs=1))
        sbpt = ctx.enter_context(tc.tile_pool(name="ptp", bufs=3))
        sbat = ctx.enter_context(tc.tile_pool(name="atn", bufs=2))
        sbnrm = ctx.enter_context(tc.tile_pool(name="nrm", bufs=2))
        sby = ctx.enter_context(tc.tile_pool(name="ysb", bufs=4))
        # PSUM: sc 2x[128,1024] = 4 banks, pv 1x[128,520] = 2 banks,
        #       tr 1x[128,128] = 1 bank, yp 1x[128,512] = 1 bank -> 8 total
        scp = ctx.enter_context(tc.tile_pool(name="psc", bufs=2, space="PSUM"))
        pvp = ctx.enter_context(tc.tile_pool(name="ppv", bufs=1, space="PSUM"))
        trp = ctx.enter_context(tc.tile_pool(name="ptr", bufs=1, space="PSUM"))
        ypp = ctx.enter_context(tc.tile_pool(name="pyp", bufs=1, space="PSUM"))

        # ---- constants ----
        bqk_sb = consts.tile([128, 4], F32, tag="bqk", name="bqk_sb")
        nc.sync.dma_start(bqk_sb[:], bqk[:, :])
        bvb_sb = consts.tile([128, 2], F32, tag="bvb", name="bvb_sb")
        nc.sync.dma_start(bvb_sb[:], bvb[:, :])
        id_sb = consts.tile([128, 128], F32R, tag="ident", name="id_sb")
        nc.sync.dma_start(id_sb[:], ident[:, :])

        # ---- input DMAs (ordered: wk, xT blocks, wv, wq, wo) ----
        wk_sb = [sbw.tile([128, C], BF16, tag=f"wk{i}", name=f"wk{i}")
                 for i in range(8)]
        wv_sb = [sbw.tile([128, C], BF16, tag=f"wv{i}", name=f"wv{i}")
                 for i in range(8)]
        wq_sb = [sbw.tile([128, C], BF16, tag=f"wq{i}", name=f"wq{i}")
                 for i in range(8)]
        xt_sb = [sbx.tile([128, S], BF16, tag=f"xt{i}", name=f"xt{i}")
                 for i in range(8)]
        for i in range(8):
            nc.sync.dma_start(wk_sb[i][:], wk[ts(i, 128), :])
        for nb in range(4):
            for i in range(8):
                nc.sync.dma_start(xt_sb[i][:, ts(nb, 512)],
                                  xT[ts(i, 128), ts(nb, 512)])
        for i in range(8):
            nc.sync.dma_start(wv_sb[i][:], wv[ts(i, 128), :])
        for i in range(8):
            nc.sync.dma_start(wq_sb[i][:], wq[ts(i, 128), :])
        wo_sb = []
        for i in range(2):
            t = sbw.tile([128, D], F32R, tag=f"wo{i}", name=f"wo{i}")
            nc.sync.dma_start(t[:], wo[ts(i, 128), :])
            wo_sb.append(t)

        # ---- persistent activations ----
        qT_sb = [sbqkv.tile([128, S], BF16, tag=f"qT{i}", name=f"qT{i}")
                 for i in range(2)]
        kT_sb = [sbqkv.tile([128, S], BF16, tag=f"kT{i}", name=f"kT{i}")
                 for i in range(2)]
        v_sb = [sbqkv.tile([128, NH * 65], BF16, tag=f"v{i}", name=f"v{i}")
                for i in range(NST)]
        aT_sb = [sbqkv.tile([128, S], F32R, tag=f"aT{i}", name=f"aT{i}")
                 for i in range(2)]

        # ones columns of v_ext (col 65h+64 = 1.0)
        for st in range(NST):
            v3 = v_sb[st][:].rearrange("p (h e) -> p h e", e=65)
            nc.gpsimd.memset(v3[:, :, 64:65], 1.0)

        # ---- emission helpers ----
        def proj_qk(which, mt, half, pool=None):
            """one [128,1024] stripe of q/k projection -> qT/kT (2 psum tiles)."""
            w_sb, dsts, bcol = {
                "q": (wq_sb, qT_sb, 0), "k": (wk_sb, kT_sb, 2)}[which]
            for h2 in range(2):
                nb = half * 2 + h2
                if pool is None:
                    pj = ypp.tile([128, 512], F32, tag="yp", name="pj")
                else:
                    pj = pool.tile([128, 512], F32, tag="sc", name="pj")
                for kt in range(8):
                    nc.tensor.matmul(
                        pj[:],
                        lhsT=w_sb[kt][:, ts(mt, 128)],
                        rhs=xt_sb[kt][:, ts(nb, 512)],
                        start=(kt == 0), stop=(kt == 7),
                    )
                nc.vector.tensor_scalar_add(
                    dsts[mt][:, ts(nb, 512)], pj[:],
                    bqk_sb[:, bcol + mt:bcol + mt + 1])

        def proj_v(st):
            """one seq-tile of v projection -> v_sb[st] (no bias)."""
            vp = ypp.tile([128, C], F32, tag="yp", name="vp_pj")
            for kt in range(8):
                nc.tensor.matmul(
                    vp[:],
                    lhsT=xt_sb[kt][:, ts(st, 128)],
                    rhs=wv_sb[kt][:],
                    start=(kt == 0), stop=(kt == 7),
                )
            v3 = v_sb[st][:].rearrange("p (h e) -> p h e", e=65)
            nc.vector.tensor_copy(
                v3[:, :, 0:64],
                vp[:].rearrange("p (h e) -> p h e", e=64))

        def transpose_at(pair, qg, qt, at_tile):
            """attn [128q,128d] -> aT[pair][:, qg*QG+qt*128 ...] (+v bias)."""
            tr = trp.tile([128, 128], F32R, tag="tr", name="tr")
            nc.tensor.transpose(tr[:], at_tile[:], id_sb[:])
            nc.vector.tensor_scalar_add(
                aT_sb[pair][:, ds(qg * QG + qt * 128, 128)], tr[:],
                bvb_sb[:, pair:pair + 1])

        def out_proj_mt(nb, mt, tail=False):
            pool = scp if tail else ypp
            yp = pool.tile([128, 512], F32, tag="sc" if tail else "yp",
                           name="yp")
            for p in range(2):
                nc.tensor.matmul(
                    yp[:],
                    lhsT=wo_sb[p][:, ts(mt, 128)],
                    rhs=aT_sb[p][:, ts(nb, 512)],
                    start=(p == 0), stop=(p == 1),
                )
            yt = sby.tile([128, 512], F32, tag="yt", name="yt")
            nc.vector.tensor_copy(yt[:], yp[:])
            nc.sync.dma_start(yT[ts(mt, 128), ts(nb, 512)], yt[:])

        # ---- interleave schedule ----
        # slots[(qg, h)][kt] -> list of thunks emitted after the scores mms
        # of iteration kt (PE program order), before exp/pv.
        slots = {(qg, h): {} for qg in range(NQG) for h in range(NH)}

        def add_slot(qg, h, kt, fn):
            slots[(qg, h)].setdefault(kt, []).append(fn)

        # v-proj st4..15 into qg0/h0 kt0..11
        for j, st in enumerate(range(4, NST)):
            add_slot(0, 0, j, (lambda st=st: proj_v(st)))
        # k-proj mt1 halves into qg0/h0 kt12..15 (needed by h2)
        add_slot(0, 0, 12, lambda: proj_qk("k", 1, 0))
        add_slot(0, 0, 14, lambda: proj_qk("k", 1, 1))
        # q-proj half1 (qg1 cols) into qg0/h1 (needed by qg1)
        add_slot(0, 1, 0, lambda: proj_qk("q", 0, 1))
        add_slot(0, 1, 4, lambda: proj_qk("q", 1, 1))

        # per-(qg,pair) attn tiles, filled by norm, consumed by transpose
        attn_tiles = {}

        def norm_pair_writes(qg, h, pva, pvb):
            pair = h // 2
            if (qg, pair) not in attn_tiles:
                attn_tiles[(qg, pair)] = [
                    sbat.tile([128, 128], F32R, tag=f"at{qt}", name=f"at{qt}")
                    for qt in range(NSB)]
            tiles = attn_tiles[(qg, pair)]
            col = 64 * (h % 2)
            # single copy per bank releases the PSUM WAR fast; recip/muls
            # then work off SBUF without blocking the next head's pv mms
            pvc = sbnrm.tile([128, 2, 4, 65], F32, tag="pvc", name="pvc")
            nc.vector.tensor_copy(
                pvc[:, 0], pva[:].rearrange("p (s e) -> p s e", e=65))
            nc.vector.tensor_copy(
                pvc[:, 1], pvb[:].rearrange("p (s e) -> p s e", e=65))
            recip = sbnrm.tile([128, 8], F32, tag="rc", name="rc")
            pc3 = pvc[:].rearrange("p a s e -> p (a s) e")
            nc.vector.reciprocal(recip[:], pc3[:, :, 64])
            for qs in range(NSB):
                nc.vector.tensor_scalar_mul(
                    tiles[qs][:, col:col + 64],
                    pc3[:, qs, 0:64],
                    recip[:, qs:qs + 1])

        def attn_head(qg, h):
            """software-pipelined: iteration j emits scores(j), pv(j-2),
            exp(j-1) so the act engine runs back-to-back."""
            pair, poff = h // 2, 64 * (h % 2)
            qt_, kt_ = qT_sb[pair], kT_sb[pair]
            # one accumulation group per PSUM bank: 4 q-subtiles per tile
            pva = pvp.tile([128, 4 * 65], F32, tag="pva", name="pva")
            pvb = pvp.tile([128, 4 * 65], F32, tag="pvb", name="pvb")
            isl = slots[(qg, h)]
            pts, scs = {}, {}
            for j in range(NST + 2):
                # exp FIRST in program order: the act engine's coalesced
                # PE-semaphore threshold then only covers work finished a
                # full period ago, so exps run back-to-back.
                kt1 = j - 1
                if 0 <= kt1 < NST:
                    pt = sbpt.tile([128, QG], BF16, tag="pt", name="pt")
                    pts[kt1] = pt
                    nc.scalar.activation(pt[:], scs.pop(kt1)[:], ACT.Exp)
                if j < NST:
                    sc = scp.tile([128, QG], F32, tag="sc", name="sc_at")
                    scs[j] = sc
                    for qb in range(2):
                        nc.tensor.matmul(
                            sc[:, ts(qb, 512)],
                            lhsT=kt_[poff:poff + 64, ts(j, 128)],
                            rhs=qt_[poff:poff + 64,
                                    ds(qg * QG + qb * 512, 512)],
                            start=True, stop=True,
                        )
                    for fn in isl.get(j, ()):
                        fn()
                kt2 = j - 2
                if kt2 >= 0:
                    ptt = pts.pop(kt2)
                    for qs in range(NSB):
                        pvt = pva if qs < 4 else pvb
                        nc.tensor.matmul(
                            pvt[:, ds((qs % 4) * 65, 65)],
                            lhsT=ptt[:, ts(qs, 128)],
                            rhs=v_sb[kt2][:, ds(65 * h, 65)],
                            start=(kt2 == 0 and qs % 4 == 0),
                            stop=(kt2 == NST - 1 and qs % 4 == 3),
                        )
            norm_pair_writes(qg, h, pva, pvb)

        # ---- phase 1: initial projections (k via the idle sc pool so the
        #      lead-in isn't serialized on the single yp buffer) ----
        proj_qk("k", 0, 0, pool=scp)
        proj_qk("k", 0, 1, pool=scp)
        for st in range(4):
            proj_v(st)
        proj_qk("q", 0, 0)
        proj_qk("q", 1, 0)

        # ---- phase 2: attention + pipelined transposes / out-proj ----
        # transposes of (qg, pair) go into slots of the following head pair;
        # out-proj of qg into slots of qg+1; tail handled explicitly.
        for qg in range(NQG):
            # transposes of previous group's pair1
            if qg > 0:
                for qt in range(NSB):
                    add_slot(qg, 0, qt, (lambda qt=qt, qg=qg:
                             transpose_at(1, qg - 1, qt,
                                          attn_tiles[(qg - 1, 1)][qt])))
                # out-proj of previous group spread over h1/h2 slots
                for j, (nb, mt) in enumerate(
                        [(2 * (qg - 1) + b, m) for b in range(2)
                         for m in range(8)]):
                    add_slot(qg, 1 + j // 16, j % 16,
                             (lambda nb=nb, mt=mt: out_proj_mt(nb, mt)))
            for h in range(NH):
                # transposes of pair0 go into h2 slots
                if h == 2:
                    for qt in range(NSB):
                        add_slot(qg, 2, qt, (lambda qt=qt, qg=qg:
                                 transpose_at(0, qg, qt,
                                              attn_tiles[(qg, 0)][qt])))
                attn_head(qg, h)

        # ---- tail: transposes of (qg1, pair1) + out-proj of qg1 ----
        qg = NQG - 1
        for half in range(2):
            for qt in range(half * 4, half * 4 + 4):
                transpose_at(1, qg, qt, attn_tiles[(qg, 1)][qt])
            for mt in range(8):
                out_proj_mt(2 * qg + half, mt, tail=True)

    nc.compile()
    return nc


def make_in_maps(x, Wq, bq, Wk, bk, Wv, bv, Wo):
    """Shard full inputs into 8 per-core input maps."""
    import ml_dtypes
    BF = ml_dtypes.bfloat16
    scale = np.float32(1.0 / np.sqrt(DH))
    xT = [np.ascontiguousarray(x[b].T).astype(BF) for b in range(2)]
    ident = np.eye(128, dtype=np.float32)
    in_maps = []
    for c in range(8):
        b, g = c // 4, c % 4
        sl = slice(C * g, C * (g + 1))
        bq_g = (bq[sl] * scale).reshape(2, 128).T
        bk_g = bk[sl].reshape(2, 128).T
        in_maps.append({
            "xT": xT[b],
            "wq": (np.ascontiguousarray(Wq[:, sl]) * scale).astype(BF),
            "wk": np.ascontiguousarray(Wk[:, sl]).astype(BF),
            "wv": np.ascontiguousarray(Wv[:, sl]).astype(BF),
            "wo": np.ascontiguousarray(Wo[sl, :]).astype(np.float32),
            "bqk": np.ascontiguousarray(
                np.concatenate([bq_g, bk_g], axis=1)).astype(np.float32),
            "bvb": np.ascontiguousarray(
                bv[sl].reshape(2, 128).T).astype(np.float32),
            "ident": ident,
        })
    return in_maps


def kernel(x, Wq, bq, Wk, bk, Wv, bv, Wo, bo):
    if os.environ.get("JAX_PLATFORMS") and \
            "axon" not in os.environ["JAX_PLATFORMS"]:
        os.environ.pop("JAX_PLATFORMS")
    trace = bool(os.environ.get("KERNEL_TRACE"))
    if trace:
        _install_ntff_shim()
    from concourse import bass_utils

    x = np.asarray(x, dtype=np.float32)
    in_maps = make_in_maps(
        x, np.asarray(Wq), np.asarray(bq), np.asarray(Wk), np.asarray(bk),
        np.asarray(Wv), np.asarray(bv), np.asarray(Wo))

    if "nc" not in _CACHE:
        _CACHE["nc"] = build_nc()
    res = bass_utils.run_bass_kernel_spmd(
        _CACHE["nc"], in_maps, core_ids=list(range(8)), trace=trace)
    _CACHE["exec_time_ns"] = res.exec_time_ns

    bo = np.asarray(bo, dtype=np.float32)
    out = np.empty((2, S, D), dtype=np.float32)
    for b in range(2):
        acc = res.results[4 * b]["yT"].copy()
        for g in range(1, 4):
            acc += res.results[4 * b + g]["yT"]
        out[b] = acc.T + bo
    return out



# revision 7
# speedup vs baseline: 1.3692x; 1.2889x over previous
"""Multi-head attention (B=2, S=2048, D=1024, H=16) on 8 TRN2 NeuronCores, v3.

Sharding: data-parallel over batch (2) x tensor-parallel over head groups
(4 groups of 4 heads).  Core c = (b = c // 4, g = c % 4).

v3 design (HAM-warm dense-PE schedule):
  - All q/k/v projections in bf16; qT/kT stored PER HEAD with the 64 dh rows
    duplicated to partitions 64-127 (dup via SBUF->SBUF DMA) so the two
    512-wide q-blocks of a scores tile run CONCURRENTLY in different PE
    row-groups (tile_position auto-derived from base partitions).
  - Projections are emitted as ~1024-cycle quarter-units and spread through
    the attention j-slots so the PE never idles -> HAM stays at K=8/8
    (2.4 GHz).  Empty late slots get dummy transposes to hold the clock.
  - Act engine runs exp back-to-back ([128,1024] per (head, kt)); it is the
    steady-state bottleneck (~1.1us/instr).
  - PV in [q, dh] orientation: lhsT = pt chunk [k,128q], rhs = v_ext [k,65]
    (64 v cols + ones col -> denominators land in pv col 64).
  - normalize = DVE reciprocal + per-partition tensor_scalar_mul; PE
    transpose puts normalized attn into aT [d, q] (+v bias folded in).
  - yT written as bf16 (halves output DMA); host accumulates in fp32.
"""

import os
import sys
import types
from contextlib import ExitStack

import numpy as np

D = 1024
S = 2048
C = 256          # head cols per core (4 heads x 64)
DH = 64
NH = 4           # heads per core
QG = 1024        # q-group width
NQG = S // QG    # 2
NST = S // 128   # 16 seq tiles
NSB = QG // 128  # 8 q-subtiles per group

_CACHE = {}


def _install_ntff_shim():
    try:
        import antenv.axon_hooks  # noqa: F401
        return
    except ImportError:
        pass
    try:
        from trn_agent_boot.trn_boot import _ntff_profile_via_ctypes
        hook = _ntff_profile_via_ctypes('/opt/axon/libaxon_pjrt.so')
    except Exception:
        hook = None
    mod = types.ModuleType('antenv.axon_hooks')
    mod.get_axon_ntff_profile_hook = lambda: hook
    mod.set_axon_ntff_profile_hook = lambda h: None
    sys.modules['antenv.axon_hooks'] = mod


def build_nc():
    import concourse.bacc as bacc
    import concourse.mybir as mybir
    import concourse.tile as tile
    from concourse.bass import ts, ds

    F32 = mybir.dt.float32
    F32R = mybir.dt.float32r
    BF16 = mybir.dt.bfloat16
    ACT = mybir.ActivationFunctionType

    nc = bacc.Bacc("TRN2", target_bir_lowering=False, debug=False)
    xT = nc.dram_tensor("xT", [D, S], BF16, kind="ExternalInput")
    wq = nc.dram_tensor("wq", [D, C], BF16, kind="ExternalInput")
    wk = nc.dram_tensor("wk", [D, C], BF16, kind="ExternalInput")
    wv = nc.dram_tensor("wv", [D, C], BF16, kind="ExternalInput")
    wo = nc.dram_tensor("wo", [C, D], F32R, kind="ExternalInput")
    bqk = nc.dram_tensor("bqk", [128, 4], F32, kind="ExternalInput")
    bvb = nc.dram_tensor("bvb", [128, 2], F32, kind="ExternalInput")
    ident = nc.dram_tensor("ident", [128, 128], F32R, kind="ExternalInput")
    yT = nc.dram_tensor("yT", [D, S], BF16, kind="ExternalOutput")

    with tile.TileContext(nc) as tc, ExitStack() as ctx:
        consts = ctx.enter_context(tc.tile_pool(name="consts", bufs=1))
        sbw = ctx.enter_context(tc.tile_pool(name="weights", bufs=1))
        sbx = ctx.enter_context(tc.tile_pool(name="xsb", bufs=1))
        sbqkv = ctx.enter_context(tc.tile_pool(name="qkv", bufs=1))
        sbpt = ctx.enter_context(tc.tile_pool(name="ptp", bufs=3))
        sbat = ctx.enter_context(tc.tile_pool(name="atn", bufs=2))
        sbnrm = ctx.enter_context(tc.tile_pool(name="nrm", bufs=2))
        sby = ctx.enter_context(tc.tile_pool(name="ysb", bufs=4))
        # PSUM: sc 2x[128,1024] = 4 banks, pv 1x(2x[128,260]) = 2 banks,
        #       tr 1x[128,128] = 1 bank, yp 1x[128,512] = 1 bank -> 8 total
        scp = ctx.enter_context(tc.tile_pool(name="psc", bufs=2, space="PSUM"))
        pvp = ctx.enter_context(tc.tile_pool(name="ppv", bufs=1, space="PSUM"))
        trp = ctx.enter_context(tc.tile_pool(name="ptr", bufs=1, space="PSUM"))
        ypp = ctx.enter_context(tc.tile_pool(name="pyp", bufs=1, space="PSUM"))

        # ---- constants ----
        bqk_sb = consts.tile([128, 4], F32, tag="bqk", name="bqk_sb")
        nc.sync.dma_start(bqk_sb[:], bqk[:, :])
        bvb_sb = consts.tile([128, 2], F32, tag="bvb", name="bvb_sb")
        nc.sync.dma_start(bvb_sb[:], bvb[:, :])
        id_sb = consts.tile([128, 128], F32R, tag="ident", name="id_sb")
        nc.sync.dma_start(id_sb[:], ident[:, :])

        # ---- input DMAs (ordered: wk, wv, x nb0, x nb1, wq, x nb2/3, wo) --
        wk_sb = [sbw.tile([128, C], BF16, tag=f"wk{i}", name=f"wk{i}")
                 for i in range(8)]
        wv_sb = [sbw.tile([128, C], BF16, tag=f"wv{i}", name=f"wv{i}")
                 for i in range(8)]
        wq_sb = [sbw.tile([128, C], BF16, tag=f"wq{i}", name=f"wq{i}")
                 for i in range(8)]
        xt_sb = [sbx.tile([128, S], BF16, tag=f"xt{i}", name=f"xt{i}")
                 for i in range(8)]
        for i in range(8):
            nc.sync.dma_start(wk_sb[i][:], wk[ts(i, 128), :])
        for i in range(8):
            nc.sync.dma_start(wv_sb[i][:], wv[ts(i, 128), :])
        for nb in range(2):
            for i in range(8):
                nc.sync.dma_start(xt_sb[i][:, ts(nb, 512)],
                                  xT[ts(i, 128), ts(nb, 512)])
        for i in range(8):
            nc.sync.dma_start(wq_sb[i][:], wq[ts(i, 128), :])
        for nb in range(2, 4):
            for i in range(8):
                nc.sync.dma_start(xt_sb[i][:, ts(nb, 512)],
                                  xT[ts(i, 128), ts(nb, 512)])
        wo_sb = []
        for i in range(2):
            t = sbw.tile([128, D], F32R, tag=f"wo{i}", name=f"wo{i}")
            nc.sync.dma_start(t[:], wo[ts(i, 128), :])
            wo_sb.append(t)

        # ---- persistent activations ----
        # per-head q/k, dh rows duplicated into partitions 64-127
        qTd_sb = [sbqkv.tile([128, S], BF16, tag=f"qTd{h}", name=f"qTd{h}")
                  for h in range(NH)]
        kTd_sb = [sbqkv.tile([128, S], BF16, tag=f"kTd{h}", name=f"kTd{h}")
                  for h in range(NH)]
        v_sb = [sbqkv.tile([128, NH * 65], BF16, tag=f"v{i}", name=f"v{i}")
                for i in range(NST)]
        aT_sb = [sbqkv.tile([128, S], F32R, tag=f"aT{i}", name=f"aT{i}")
                 for i in range(2)]

        # ones columns of v_ext (col 65h+64 = 1.0)
        for st in range(NST):
            v3 = v_sb[st][:].rearrange("p (h e) -> p h e", e=65)
            nc.gpsimd.memset(v3[:, :, 64:65], 1.0)

        # ---- projection quarter-units (~1024 PE cycles each) ----
        open_pj = {}

        def proj_unit(which, mt, nb, u, pool=None):
            """2 of the 8 k-tile matmuls of one [128,512] q/k proj stripe;
            u==3 finishes with per-head bias-add + row-dup DMA."""
            w_sb, dsts, bcol = {
                "q": (wq_sb, qTd_sb, 0), "k": (wk_sb, kTd_sb, 2)}[which]
            key = (which, mt, nb)
            if u == 0:
                p = pool if pool is not None else ypp
                open_pj[key] = p.tile([128, 512], F32,
                                      tag="sc" if p is scp else "yp",
                                      name=f"pj_{which}{mt}{nb}")
            pj = open_pj[key]
            for kt in range(2 * u, 2 * u + 2):
                nc.tensor.matmul(
                    pj[:],
                    lhsT=w_sb[kt][:, ts(mt, 128)],
                    rhs=xt_sb[kt][:, ts(nb, 512)],
                    start=(kt == 0), stop=(kt == 7),
                )
            if u == 3:
                for hh in range(2):
                    head = 2 * mt + hh
                    dst = dsts[head]
                    nc.vector.tensor_scalar_add(
                        dst[64 * hh:64 * hh + 64, ts(nb, 512)],
                        pj[64 * hh:64 * hh + 64, :],
                        bqk_sb[64 * hh:64 * hh + 64,
                               bcol + mt:bcol + mt + 1])
                    # duplicate dh rows to the other partition half
                    nc.sync.dma_start(
                        dst[64 * (1 - hh):64 * (1 - hh) + 64, ts(nb, 512)],
                        dst[64 * hh:64 * hh + 64, ts(nb, 512)])
                del open_pj[key]

        open_vp = {}

        def proj_v_half(st, half, pool=None):
            """half a seq-tile of v projection (4 of 8 k-tiles)."""
            if half == 0:
                p = pool if pool is not None else ypp
                open_vp[st] = p.tile([128, C], F32,
                                     tag="sc" if p is scp else "yp",
                                     name=f"vp{st}")
            vp = open_vp[st]
            for kt in range(4 * half, 4 * half + 4):
                nc.tensor.matmul(
                    vp[:],
                    lhsT=xt_sb[kt][:, ts(st, 128)],
                    rhs=wv_sb[kt][:],
                    start=(kt == 0), stop=(kt == 7),
                )
            if half == 1:
                v3 = v_sb[st][:].rearrange("p (h e) -> p h e", e=65)
                nc.vector.tensor_copy(
                    v3[:, :, 0:64],
                    vp[:].rearrange("p (h e) -> p h e", e=64))
                del open_vp[st]

        def transpose_at(pair, qg, qt, at_tile):
            """attn [128q,128d] -> aT[pair][:, qg*QG+qt*128 ...] (+v bias)."""
            tr = trp.tile([128, 128], F32R, tag="tr", name="tr")
            nc.tensor.transpose(tr[:], at_tile[:], id_sb[:])
            nc.vector.tensor_scalar_add(
                aT_sb[pair][:, ds(qg * QG + qt * 128, 128)], tr[:],
                bvb_sb[:, pair:pair + 1])

        def transpose_dummy():
            """PE filler to keep the HAM clock-gate open in empty slots."""
            tr = trp.tile([128, 128], F32R, tag="tr", name="trd")
            nc.tensor.transpose(tr[:], id_sb[:], id_sb[:])

        open_yp = {}

        def out_proj_half(nb, mt, p, tail=False):
            """one of the two accumulation matmuls of an out-proj stripe."""
            key = (nb, mt)
            if p == 0:
                pool = scp if tail else ypp
                open_yp[key] = pool.tile([128, 512], F32,
                                         tag="sc" if tail else "yp",
                                         name=f"yp{nb}{mt}")
            yp = open_yp[key]
            nc.tensor.matmul(
                yp[:],
                lhsT=wo_sb[p][:, ts(mt, 128)],
                rhs=aT_sb[p][:, ts(nb, 512)],
                start=(p == 0), stop=(p == 1),
            )
            if p == 1:
                yt = sby.tile([128, 512], BF16, tag="yt", name="yt")
                nc.vector.tensor_copy(yt[:], yp[:])
                nc.sync.dma_start(yT[ts(mt, 128), ts(nb, 512)], yt[:])
                del open_yp[key]

        # ---- interleave schedule ----
        slots = {(qg, h): {} for qg in range(NQG) for h in range(NH)}

        def add_slot(qg, h, kt, fn):
            slots[(qg, h)].setdefault(kt, []).append(fn)

        # per-(qg,pair) attn tiles, filled by norm, consumed by transpose
        attn_tiles = {}

        def norm_pair_writes(qg, h, pva, pvb):
            pair = h // 2
            if (qg, pair) not in attn_tiles:
                attn_tiles[(qg, pair)] = [
                    sbat.tile([128, 128], F32R, tag=f"at{qt}", name=f"at{qt}")
                    for qt in range(NSB)]
            tiles = attn_tiles[(qg, pair)]
            col = 64 * (h % 2)
            pvc = sbnrm.tile([128, 2, 4, 65], F32, tag="pvc", name="pvc")
            nc.vector.tensor_copy(
                pvc[:, 0], pva[:].rearrange("p (s e) -> p s e", e=65))
            nc.vector.tensor_copy(
                pvc[:, 1], pvb[:].rearrange("p (s e) -> p s e", e=65))
            recip = sbnrm.tile([128, 8], F32, tag="rc", name="rc")
            pc3 = pvc[:].rearrange("p a s e -> p (a s) e")
            nc.vector.reciprocal(recip[:], pc3[:, :, 64])
            for qs in range(NSB):
                nc.vector.tensor_scalar_mul(
                    tiles[qs][:, col:col + 64],
                    pc3[:, qs, 0:64],
                    recip[:, qs:qs + 1])

        def attn_head(qg, h):
            """software-pipelined: iteration j emits scores(j), pv(j-2),
            exp(j-1) so the act engine runs back-to-back."""
            ktd, qtd = kTd_sb[h], qTd_sb[h]
            pva = pvp.tile([128, 4 * 65], F32, tag="pva", name="pva")
            pvb = pvp.tile([128, 4 * 65], F32, tag="pvb", name="pvb")
            isl = slots[(qg, h)]
            pts, scs = {}, {}
            for j in range(NST + 2):
                kt1 = j - 1
                if 0 <= kt1 < NST:
                    pt = sbpt.tile([128, QG], BF16, tag="pt", name="pt")
                    pts[kt1] = pt
                    nc.scalar.activation(pt[:], scs.pop(kt1)[:], ACT.Exp)
                if j < NST:
                    sc = scp.tile([128, QG], F32, tag="sc", name="sc_at")
                    scs[j] = sc
                    # two q-blocks in different PE row groups -> concurrent
                    for qb in range(2):
                        rg = 64 * qb
                        nc.tensor.matmul(
                            sc[:, ts(qb, 512)],
                            lhsT=ktd[rg:rg + 64, ts(j, 128)],
                            rhs=qtd[rg:rg + 64,
                                    ds(qg * QG + qb * 512, 512)],
                            start=True, stop=True,
                        )
                    for fn in isl.get(j, ()):
                        fn()
                kt2 = j - 2
                if kt2 >= 0:
                    ptt = pts.pop(kt2)
                    for qs in range(NSB):
                        pvt = pva if qs < 4 else pvb
                        nc.tensor.matmul(
                            pvt[:, ds((qs % 4) * 65, 65)],
                            lhsT=ptt[:, ts(qs, 128)],
                            rhs=v_sb[kt2][:, ds(65 * h, 65)],
                            start=(kt2 == 0 and qs % 4 == 0),
                            stop=(kt2 == NST - 1 and qs % 4 == 3),
                        )
            norm_pair_writes(qg, h, pva, pvb)

        # ---- lead-in: minimal projections, DMA-arrival ordered ----
        # nb0-gated: v st0-3, k mt0 nb0; then wq+nb1: q mt0 nb0, v st4-7,
        # k/q nb1; then k nb2, nb3.  Alternate psum pool ypp/trp so drains
        # overlap next group's matmuls.
        _pools = [ypp, scp]
        _pi = 0

        def _nextpool():
            nonlocal _pi
            _pi ^= 1
            return _pools[_pi]

        for st in range(4):
            p = _nextpool()
            proj_v_half(st, 0, pool=p)
            proj_v_half(st, 1, pool=p)
        p = _nextpool()
        for u in range(4):
            proj_unit("k", 0, 0, u, pool=p)
        p = _nextpool()
        for u in range(4):
            proj_unit("q", 0, 0, u, pool=p)
        for st in range(4, 8):
            p = _nextpool()
            proj_v_half(st, 0, pool=p)
            proj_v_half(st, 1, pool=p)
        for nb in (1,):
            p = _nextpool()
            for u in range(4):
                proj_unit("k", 0, nb, u, pool=p)
            p = _nextpool()
            for u in range(4):
                proj_unit("q", 0, nb, u, pool=p)
        for nb in (2, 3):
            p = _nextpool()
            for u in range(4):
                proj_unit("k", 0, nb, u, pool=p)

        # ---- slot fillers ----
        # (0,0): v st8-15 (both halves per slot) on j0-7; q mt1 nb0/nb1 j8-15
        for j in range(8):
            st = 8 + j
            add_slot(0, 0, j, (lambda st=st: proj_v_half(st, 0)))
            add_slot(0, 0, j, (lambda st=st: proj_v_half(st, 1)))
        for j in range(8):
            nb, u = j // 4, j % 4
            add_slot(0, 0, 8 + j, (lambda nb=nb, u=u:
                                   proj_unit("q", 1, nb, u)))
        # (0,1): k mt1 all four nb
        for j in range(16):
            nb, u = j // 4, j % 4
            add_slot(0, 1, j, (lambda nb=nb, u=u:
                               proj_unit("k", 1, nb, u)))
        # (0,2): transposes pair0 qg0 on j0-7; q mt0 nb2/nb3 on j8-15
        for qt in range(NSB):
            add_slot(0, 2, qt, (lambda qt=qt:
                     transpose_at(0, 0, qt, attn_tiles[(0, 0)][qt])))
        for j in range(8):
            nb, u = 2 + j // 4, j % 4
            add_slot(0, 2, 8 + j, (lambda nb=nb, u=u:
                                   proj_unit("q", 0, nb, u)))
        # (0,3): q mt1 nb2/nb3 on j0-7; dummies j8-15
        for j in range(8):
            nb, u = 2 + j // 4, j % 4
            add_slot(0, 3, j, (lambda nb=nb, u=u:
                               proj_unit("q", 1, nb, u)))
        for j in range(8, 16):
            add_slot(0, 3, j, transpose_dummy)
        # (1,0): transposes pair1 qg0 on j0-7; out-proj nb0 mt0-3 j8-15
        for qt in range(NSB):
            add_slot(1, 0, qt, (lambda qt=qt:
                     transpose_at(1, 0, qt, attn_tiles[(0, 1)][qt])))
        for j in range(8):
            mt, p = j // 2, j % 2
            add_slot(1, 0, 8 + j, (lambda mt=mt, p=p:
                                   out_proj_half(0, mt, p)))
        # (1,1): out-proj nb0 mt4-7, nb1 mt0-3
        for j in range(8):
            mt, p = 4 + j // 2, j % 2
            add_slot(1, 1, j, (lambda mt=mt, p=p:
                               out_proj_half(0, mt, p)))
        for j in range(8):
            mt, p = j // 2, j % 2
            add_slot(1, 1, 8 + j, (lambda mt=mt, p=p:
                                   out_proj_half(1, mt, p)))
        # (1,2): transposes pair0 qg1 j0-7; out-proj nb1 mt4-7 j8-15
        for qt in range(NSB):
            add_slot(1, 2, qt, (lambda qt=qt:
                     transpose_at(0, 1, qt, attn_tiles[(1, 0)][qt])))
        for j in range(8):
            mt, p = 4 + j // 2, j % 2
            add_slot(1, 2, 8 + j, (lambda mt=mt, p=p:
                                   out_proj_half(1, mt, p)))
        # (1,3): dummies to keep PE warm
        for j in range(16):
            add_slot(1, 3, j, transpose_dummy)

        # ---- attention ----
        for qg in range(NQG):
            for h in range(NH):
                attn_head(qg, h)

        # ---- tail: transposes of (qg1, pair1) + out-proj of qg1 ----
        for half in range(2):
            for qt in range(half * 4, half * 4 + 4):
                transpose_at(1, 1, qt, attn_tiles[(1, 1)][qt])
            for mt in range(8):
                for p in range(2):
                    out_proj_half(2 + half, mt, p, tail=True)

    nc.compile()
    return nc


def make_in_maps(x, Wq, bq, Wk, bk, Wv, bv, Wo):
    """Shard full inputs into 8 per-core input maps."""
    import ml_dtypes
    BF = ml_dtypes.bfloat16
    scale = np.float32(1.0 / np.sqrt(DH))
    xT = [np.ascontiguousarray(x[b].T).astype(BF) for b in range(2)]
    ident = np.eye(128, dtype=np.float32)
    in_maps = []
    for c in range(8):
        b, g = c // 4, c % 4
        sl = slice(C * g, C * (g + 1))
        bq_g = (bq[sl] * scale).reshape(2, 128).T
        bk_g = bk[sl].reshape(2, 128).T
        in_maps.append({
            "xT": xT[b],
            "wq": (np.ascontiguousarray(Wq[:, sl]) * scale).astype(BF),
            "wk": np.ascontiguousarray(Wk[:, sl]).astype(BF),
            "wv": np.ascontiguousarray(Wv[:, sl]).astype(BF),
            "wo": np.ascontiguousarray(Wo[sl, :]).astype(np.float32),
            "bqk": np.ascontiguousarray(
                np.concatenate([bq_g, bk_g], axis=1)).astype(np.float32),
            "bvb": np.ascontiguousarray(
                bv[sl].reshape(2, 128).T).astype(np.float32),
            "ident": ident,
        })
    return in_maps


def kernel(x, Wq, bq, Wk, bk, Wv, bv, Wo, bo):
    if os.environ.get("JAX_PLATFORMS") and \
            "axon" not in os.environ["JAX_PLATFORMS"]:
        os.environ.pop("JAX_PLATFORMS")
    trace = bool(os.environ.get("KERNEL_TRACE"))
    if trace:
        _install_ntff_shim()
    from concourse import bass_utils

    x = np.asarray(x, dtype=np.float32)
    in_maps = make_in_maps(
        x, np.asarray(Wq), np.asarray(bq), np.asarray(Wk), np.asarray(bk),
        np.asarray(Wv), np.asarray(bv), np.asarray(Wo))

    if "nc" not in _CACHE:
        _CACHE["nc"] = build_nc()
    res = bass_utils.run_bass_kernel_spmd(
        _CACHE["nc"], in_maps, core_ids=list(range(8)), trace=trace)
    _CACHE["exec_time_ns"] = res.exec_time_ns

    bo = np.asarray(bo, dtype=np.float32)
    out = np.empty((2, S, D), dtype=np.float32)
    for b in range(2):
        acc = res.results[4 * b]["yT"].astype(np.float32)
        for g in range(1, 4):
            acc += res.results[4 * b + g]["yT"].astype(np.float32)
        out[b] = acc.T + bo
    return out
